# revision 50
# baseline (speedup 1.0000x reference)
"""Multi-head causal self-attention (GPT-style block) on 8 Trainium2 NeuronCores.

Data-parallel over batch (B=8 -> 1 element/core), weights replicated.

v3 design (v2 measured 181.8us):
- Bias algebra: k-bias dropped (softmax row-shift invariance), v-bias folded
  into b_proj on host, q-bias added per-partition in the q evacuation.
- Scores are K=128 with the inactive half of qTp zeroed (true K=64 PE row
  tiling via tile_position miscompiles on HW: CoreSim-correct, garbage on
  device -- do not retry).
- Fast normalization: av psum evacuated to SBUF in one fp32 copy (frees the
  bank that gates the next head pair's AV), reciprocal of just the ones-row,
  bf16 cast, rank-1 matmul broadcast of 1/d, one DVE mul av_sb*bp -> oT
  (single psum operand; two-psum-operand TensorTensor is rejected by the
  BIR verifier, as are partition-broadcast DMAs and partition-strided ops).
- ScalarE: exp + a few early evacs; DVE: psum evacs/normalize; Pool: masks.
- qg0+qg1 attention interleaved per head pair with qkv/proj matmuls as PE
  filler; PE warm-up junk matmuls + multi-queue DMA issue shrink the head.
"""

import numpy as np

import concourse.bass as bass
import concourse.mybir as mybir
import concourse.tile as tile
from concourse import bacc, bass_utils
from concourse.masks import make_identity, make_upper_triangular

F32 = mybir.dt.float32
F32R = mybir.dt.float32r
BF16 = mybir.dt.bfloat16
EXP = mybir.ActivationFunctionType.Exp

T = 1024
H = 768
NH = 12
HS = 64
TT = T // 128   # 8 token tiles
FT = H // 128   # 6 feature tiles
NP = NH // 2    # 6 head pairs
N_CORES = 8

PAIRS = {0: [(0, 1), (2, 3)], 1: [(0, 1), (2, 3), (4, 5), (6, 7)]}


def _pair_geom(qg, kt0, kt1):
    off0 = max(128 * kt0, 512 * qg)
    off1 = max(128 * kt1, 512 * qg)
    return off0, 512 * (qg + 1) - off0, off1, 512 * (qg + 1) - off1


def build():
    nc = bacc.Bacc(None, target_bir_lowering=False)

    x_d = nc.dram_tensor("x", [T, H], BF16, kind="ExternalInput")
    wa_d = nc.dram_tensor("W_attn", [H, 3 * H], BF16, kind="ExternalInput")
    bq_d = nc.dram_tensor("bq", [128, NP], F32, kind="ExternalInput")
    wp_d = nc.dram_tensor("W_proj", [H, H], BF16, kind="ExternalInput")
    bp_d = nc.dram_tensor("bp", [H], BF16, kind="ExternalInput")
    y_d = nc.dram_tensor("y", [T, H], BF16, kind="ExternalOutput")

    with tile.TileContext(nc) as tc:
        with (
            tc.tile_pool(name="sb", bufs=1) as sb,
            tc.tile_pool(name="ps", bufs=1, space="PSUM") as ps,
        ):
            # ---------------- persistent SBUF ----------------
            wat = sb.tile([128, FT, 3 * H], BF16, tag="wat")
            wpr = sb.tile([128, FT, H], BF16, tag="wpr")
            x_bf = sb.tile([128, TT, H], BF16, tag="x_bf")
            xT = sb.tile([128, FT, T], BF16, tag="xT")
            qTp = sb.tile([128, NH, T], BF16, tag="qTp")
            kT = sb.tile([128, NP, T], BF16, tag="kT")
            v_bf = sb.tile([128, TT, NH * (HS + 1) + 64], BF16, tag="v_bf")
            oT = sb.tile([128, FT, T], BF16, tag="oT")
            bqc = sb.tile([128, NP], F32, tag="bqc")
            bp_rowb = sb.tile([1, H], BF16, tag="bp_rowb")
            ones0 = sb.tile([1, 128], BF16, tag="ones0")
            ones_all = sb.tile([65, 64], BF16, tag="ones_all")  # row 64 = 1.0
            tri = sb.tile([128, 128], BF16, tag="tri")
            ident = sb.tile([128, 128], BF16, tag="ident")
            warm = sb.tile([128, 512], BF16, tag="warm")
            junks = sb.tile([1, 16], F32, tag="junks")

            # ---------------- consts (pool engine, front of its queue) ----
            nc.vector.memset(warm[:], 0.125)
            make_identity(nc, ident[:])
            make_upper_triangular(nc, tri[:], val=1.0, diag=True)
            nc.gpsimd.memset(ones0[:], 1.0)
            nc.gpsimd.memset(ones_all[:], 1.0)
            nc.gpsimd.memset(v_bf[:, :, NH * (HS + 1):], 0.0)
            nc.gpsimd.memset(v_bf[:, :, HS:NH * (HS + 1):HS + 1], 1.0)
            nc.gpsimd.memset(qTp[64:128, 0:NH:2, :], 0.0)

            # ---------------- PE warm-up (HAM) while DMAs land ------------
            with nc.named_scope("head"):
                pw = ps.tile([128, 512], F32, tag="av", bufs=2, name="pw")
                for _ in range(24):
                    nc.tensor.matmul(pw[:], warm[:, :128], warm[:], start=True, stop=True)
                # ACT exp table preload
                nc.scalar.activation(junks[:], warm[:1, :16], EXP, scale=0.125)

            # ---------------- DMA issue, spread across queues -------------
            # priority: x tiles 0-3 and the qk weights gate the first scores;
            # x tiles 4-7 are only needed for the second token group.
            for tt in range(4):
                eng = nc.sync if tt % 2 == 0 else nc.scalar
                eng.dma_start(x_bf[:, tt, :], x_d[tt * 128:(tt + 1) * 128, :])
            for ft in range(FT):
                eng = nc.scalar if ft % 2 == 0 else nc.sync
                eng.dma_start(wat[:, ft, :2 * H], wa_d[ft * 128:(ft + 1) * 128, :2 * H])
            for tt in range(4, TT):
                eng = nc.sync if tt % 2 == 0 else nc.scalar
                eng.dma_start(x_bf[:, tt, :], x_d[tt * 128:(tt + 1) * 128, :])
            nc.gpsimd.dma_start(bqc[:], bq_d[:, :])
            for ft in range(FT):
                nc.gpsimd.dma_start(wat[:, ft, 2 * H:], wa_d[ft * 128:(ft + 1) * 128, 2 * H:])
            for ft in range(FT):
                nc.gpsimd.dma_start(wpr[:, ft, :], wp_d[ft * 128:(ft + 1) * 128, :])
            nc.gpsimd.dma_start(bp_rowb[:], bp_d[None, :])

            # ---------------- emission helpers ----------------------------
            def emit_qk(hp, which, tgs=(0, 1)):
                """q (nt=hp) or k (nt=6+hp) projection -> qTp/kT, DVE evac.
                q-bias is fused into the DVE evac as a per-partition add."""
                nt = hp if which == "q" else NP + hp
                for tg in tgs:
                    pq = ps.tile([128, 512], F32, tag="op", bufs=2, name="pq")
                    for ft in range(FT):
                        nc.tensor.matmul(
                            pq[:],
                            wat[:, ft, nt * 128:(nt + 1) * 128],
                            xT[:, ft, tg * 512:(tg + 1) * 512],
                            start=(ft == 0),
                            stop=(ft == FT - 1),
                        )
                    sl = slice(tg * 512, (tg + 1) * 512)
                    if which == "q":
                        nc.vector.tensor_scalar_add(
                            qTp[0:64, 2 * hp, sl], pq[0:64, :], bqc[0:64, hp:hp + 1])
                        nc.vector.tensor_scalar_add(
                            qTp[64:128, 2 * hp + 1, sl], pq[64:128, :],
                            bqc[64:128, hp:hp + 1])
                    else:
                        nc.vector.tensor_copy(kT[:, hp, sl], pq[:])

            def emit_vmm(tt, early):
                """v projection for one token tile -> v_pl."""
                for ng in range(2):
                    pv = ps.tile([128, 512], F32, tag="op", bufs=2, name="pv")
                    for ft in range(FT):
                        nc.tensor.matmul(
                            pv[:, :384],
                            xT[:, ft, tt * 128:(tt + 1) * 128],
                            wat[:, ft, 2 * H + 384 * ng: 2 * H + 384 * (ng + 1)],
                            start=(ft == 0),
                            stop=(ft == FT - 1),
                        )
                    dst = v_bf[:, tt, 390 * ng:390 * (ng + 1)].rearrange(
                        "p (h d) -> p h d", d=65)[:, :, :64]
                    srcv = pv[:, :384].rearrange("p (h d) -> p h d", d=64)
                    # early tiles: ACT is idle pre-attn; later ones: DVE
                    if early:
                        nc.scalar.copy(dst, srcv)
                    else:
                        nc.vector.tensor_copy(dst, srcv)

            def emit_scores_pair(hp, qg, pi, pts):
                """scores + exp + mask for one kt pair. Both heads' scores
                land in ONE psum tile so exp is a single wide ACT instruction
                and each mask mul covers both heads."""
                kt0, kt1 = PAIRS[qg][pi]
                off0, w0, off1, w1 = _pair_geom(qg, kt0, kt1)
                sps = [
                    ps.tile([128, 1024], F32, tag=f"s{hi}", bufs=1, name=f"sp{hi}")
                    for hi in range(2)
                ]
                for kt, off, w, so in ((kt0, off0, w0, 0), (kt1, off1, w1, w0)):
                    for hi in range(2):
                        nc.tensor.matmul(
                            sps[hi][:, so:so + w],
                            kT[:, hp, kt * 128:(kt + 1) * 128],
                            qTp[:, 2 * hp + hi, off:off + w],
                            start=True,
                            stop=True,
                        )
                vw = w0 + w1
                diag = 128 * kt0 >= 512 * qg
                for hi in range(2):
                    nc.scalar.activation(
                        pts[:, pi, hi, :vw], sps[hi][:, :vw], EXP, scale=0.125)
                    if diag:
                        nc.gpsimd.tensor_mul(
                            pts[:, pi, hi, :128], pts[:, pi, hi, :128],
                            tri[:])
                        nc.gpsimd.tensor_mul(
                            pts[:, pi, hi, w0:w0 + 128],
                            pts[:, pi, hi, w0:w0 + 128], tri[:])

            def emit_av(hp, qg, pts, avs, pis, first, last):
                """AV accumulation for pair indices pis of (hp, qg)."""
                for hi in range(2):
                    h = 2 * hp + hi
                    for pi in pis:
                        kt0, kt1 = PAIRS[qg][pi]
                        off0, w0, off1, w1 = _pair_geom(qg, kt0, kt1)
                        for kt, off, w, so in ((kt0, off0, w0, 0), (kt1, off1, w1, w0)):
                            nc.tensor.matmul(
                                avs[hi][:65, off - 512 * qg: off - 512 * qg + w],
                                v_bf[:, kt, 65 * h:65 * h + 65],
                                pts[:, pi, hi, so:so + w],
                                start=(first and pi == pis[0] and so == 0),
                                stop=(last and pi == pis[-1] and so == w0),
                            )

            def emit_recip(avs, on_act=False):
                """Evacuate av psum -> fp32 SBUF (frees the bank fast, it
                gates the next head pair's AV), reciprocal of the whole tile
                (a base-64 single-partition custom DVE op NaNs on HW; rows
                0-63 are unused junk), bf16 cast of the denominator row.
                on_act moves the copies to ACT (used in the flush, where ACT
                has no exp work and the DVE is the serializer)."""
                avsbs = []
                recbs = []
                for hi in range(2):
                    avsb = sb.tile([65, 512], F32, tag="avsb", bufs=4, name="avsb")
                    rec = sb.tile([65, 512], F32, tag="rec", bufs=3, name="rec")
                    recb = sb.tile([65, 512], BF16, tag="recb", bufs=3, name="recb")
                    if on_act:
                        nc.scalar.copy(avsb[:, :], avs[hi][:65, :])
                    else:
                        nc.vector.tensor_copy(avsb[:, :], avs[hi][:65, :])
                    nc.vector.reciprocal_approx_fast(rec[:, :], avsb[:, :])
                    if on_act:
                        nc.scalar.copy(recb[64:65, :], rec[64:65, :])
                    else:
                        nc.vector.tensor_copy(recb[64:65, :], rec[64:65, :])
                    avsbs.append(avsb)
                    recbs.append(recb)
                return (avsbs, recbs)

            def emit_norm(hp, qg, avs, r, hi):
                """broadcast 1/d to 64 partitions with a bf16 rank-1 matmul,
                then one DVE mul av_sb * bp -> oT (single psum operand)."""
                avsbs, recbs = r
                bp = ps.tile([128, 512], F32, tag="op", bufs=2, name="bp")
                nc.tensor.matmul(
                    bp[:64, :],
                    ones_all[64:65, :64],
                    recbs[hi][64:65, :],
                    start=True, stop=True,
                )
                dst = slice(512 * qg, 512 * (qg + 1))
                if hi == 0:
                    nc.vector.tensor_mul(oT[:64, hp, dst], avsbs[0][:64, :], bp[:64, :])
                else:
                    sc = sb.tile([64, 512], BF16, tag="sc", bufs=4, name="sc")
                    nc.vector.tensor_mul(sc[:], avsbs[1][:64, :], bp[:64, :])
                    nc.sync.dma_start(oT[64:128, hp, dst], sc[:])

            def emit_proj(tt):
                ysb = sb.tile([128, H], BF16, tag="ysb", bufs=4, name="ysb")
                for ng in range(2):
                    py = ps.tile([128, 512], F32, tag="op", bufs=2, name="py")
                    for ft in range(FT):
                        nc.tensor.matmul(
                            py[:, :384],
                            oT[:, ft, tt * 128:(tt + 1) * 128],
                            wpr[:, ft, 384 * ng:384 * (ng + 1)],
                            start=(ft == 0),
                            stop=False,
                        )
                    nc.tensor.matmul(
                        py[:, :384],
                        ones0[:1, :],
                        bp_rowb[:1, 384 * ng:384 * (ng + 1)],
                        start=False,
                        stop=True,
                    )
                    if ng == 0:
                        nc.scalar.copy(ysb[:, :384], py[:, :384])
                    else:
                        nc.vector.tensor_copy(ysb[:, 384:], py[:, :384])
                    nc.sync.dma_start(
                        y_d[tt * 128:(tt + 1) * 128, 384 * ng:384 * (ng + 1)],
                        ysb[:, 384 * ng:384 * (ng + 1)])

            # ---------------- x transpose (evac on DVE) -------------------
            # qk for token group 0 only needs x tiles 0-3: emit it mid-loop
            # so the first scores can start while x tiles 4-7 transpose.
            with nc.named_scope("xT"):
                for tt in range(TT):
                    pt = ps.tile([128, FT * 128], BF16, tag="op", bufs=2, name="pt")
                    for ft in range(FT):
                        nc.tensor.transpose(
                            pt[:, ft * 128:(ft + 1) * 128],
                            x_bf[:, tt, ft * 128:(ft + 1) * 128],
                            ident[:],
                        )
                    if tt % 2 == 0:
                        nc.vector.tensor_copy(
                            xT[:, :, tt * 128:(tt + 1) * 128],
                            pt[:].rearrange("p (f t) -> p f t", t=128),
                        )
                    else:
                        nc.scalar.copy(
                            xT[:, :, tt * 128:(tt + 1) * 128],
                            pt[:].rearrange("p (f t) -> p f t", t=128),
                        )
                    if tt == 3:
                        nc.vector.memset(qTp[0:64, 1:2, :], 0.0)
                        emit_qk(0, "q", tgs=(0,))
                        emit_qk(0, "k", tgs=(0,))

            # ---------------- main pipeline -------------------------------
            # step hp: scores(hp) both qg, AV/recip/norm(hp-1), interleaved
            # with qk/t/v/etapply fillers between dependent chunks.
            with nc.named_scope("attn"):
                state = {}
                for hp in range(NP + 1):
                    prev = state.get(hp - 1)
                    if prev is not None:
                        prev["av0"] = [
                            ps.tile([128, 512], F32, tag="av", bufs=2, name="av0")
                            for _ in range(2)
                        ]
                    if hp < NP:
                        pts0 = sb.tile([128, 2, 2, 1024], BF16, tag="pA", bufs=2,
                                       name="ptsA")
                        pts = sb.tile([128, 4, 2, 1024], BF16, tag="p", bufs=2,
                                      name="pts")
                        state[hp] = {"pts": pts, "pts0": pts0}
                        if prev is not None:
                            emit_av(hp - 1, 0, prev["pts0"], prev["av0"], [0],
                                    first=True, last=False)
                        prev2 = state.get(hp - 2)
                        if prev2 is not None:
                            emit_norm(hp - 2, 1, prev2["av1"], prev2["r1"], 0)
                            emit_norm(hp - 2, 1, prev2["av1"], prev2["r1"], 1)
                        emit_scores_pair(hp, 0, 0, pts0)
                        if hp == 0:
                            emit_qk(0, "q", tgs=(1,))
                            emit_vmm(0, early=False)
                            emit_vmm(1, early=False)
                        if prev is not None:
                            emit_av(hp - 1, 0, prev["pts0"], prev["av0"], [1],
                                    first=False, last=True)
                        emit_scores_pair(hp, 0, 1, pts0)
                        if hp == 0:
                            emit_qk(0, "k", tgs=(1,))
                            emit_vmm(2, early=False)
                            emit_vmm(3, early=False)
                        if prev is not None:
                            prev["r0"] = emit_recip(prev["av0"], on_act=True)
                            prev["av1"] = [
                                ps.tile([128, 512], F32, tag="av", bufs=2, name="av1")
                                for _ in range(2)
                            ]
                        emit_scores_pair(hp, 1, 0, pts)
                        if hp == 0:
                            for tt in range(4, TT):
                                emit_vmm(tt, early=False)
                            nc.vector.memset(qTp[0:64, 3:NH:2, :], 0.0)
                        if prev is not None:
                            emit_av(hp - 1, 1, prev["pts"], prev["av1"], [0],
                                    first=True, last=False)
                            emit_norm(hp - 1, 0, prev["av0"], prev["r0"], 0)
                        emit_scores_pair(hp, 1, 1, pts)
                        if prev is not None:
                            emit_av(hp - 1, 1, prev["pts"], prev["av1"], [1],
                                    first=False, last=False)
                            emit_norm(hp - 1, 0, prev["av0"], prev["r0"], 1)
                        emit_scores_pair(hp, 1, 2, pts)
                        if hp + 1 < NP:
                            emit_qk(hp + 1, "q")
                        if prev is not None:
                            emit_av(hp - 1, 1, prev["pts"], prev["av1"], [2],
                                    first=False, last=False)
                        emit_scores_pair(hp, 1, 3, pts)
                        if hp + 1 < NP:
                            emit_qk(hp + 1, "k")
                        if prev is not None:
                            emit_av(hp - 1, 1, prev["pts"], prev["av1"], [3],
                                    first=False, last=True)
                            prev["r1"] = emit_recip(prev["av1"], on_act=True)
                    else:
                        # flush last head pair: keep the PE stream dense
                        # (av0, av1, proj0..3) while the cheap norm chains
                        # drain on DVE in parallel.
                        prev2 = state.get(hp - 2)
                        emit_av(hp - 1, 0, prev["pts0"], prev["av0"], [0, 1],
                                first=True, last=True)
                        prev["r0"] = emit_recip(prev["av0"], on_act=True)
                        if prev2 is not None:
                            emit_norm(hp - 2, 1, prev2["av1"], prev2["r1"], 0)
                            emit_norm(hp - 2, 1, prev2["av1"], prev2["r1"], 1)
                        prev["av1"] = [
                            ps.tile([128, 512], F32, tag="av", bufs=2, name="av1")
                            for _ in range(2)
                        ]
                        emit_av(hp - 1, 1, prev["pts"], prev["av1"], [0, 1],
                                first=True, last=False)
                        emit_norm(hp - 1, 0, prev["av0"], prev["r0"], 0)
                        emit_norm(hp - 1, 0, prev["av0"], prev["r0"], 1)
                        emit_av(hp - 1, 1, prev["pts"], prev["av1"], [2, 3],
                                first=False, last=True)
                        prev["r1"] = emit_recip(prev["av1"], on_act=True)
                        emit_proj(0)
                        emit_proj(1)
                        emit_norm(hp - 1, 1, prev["av1"], prev["r1"], 0)
                        emit_norm(hp - 1, 1, prev["av1"], prev["r1"], 1)
                        emit_proj(2)
                        emit_proj(3)

            # ---------------- output projection ---------------------------
            with nc.named_scope("proj"):
                for tt in range(4, TT):
                    emit_proj(tt)

    nc.compile()
    return nc


_NC = None


def _run(in_maps, trace=False, **kwargs):
    global _NC
    if _NC is None:
        _NC = build()
    return bass_utils.run_bass_kernel_spmd(
        _NC, in_maps, core_ids=list(range(N_CORES)), trace=trace, **kwargs
    )


def make_in_maps(x, W_attn, b_attn, W_proj, b_proj):
    import ml_dtypes
    bf = ml_dtypes.bfloat16
    x = np.asarray(x, dtype=np.float32).astype(bf)
    W_attn_f = np.asarray(W_attn, dtype=np.float32)
    b_attn_f = np.asarray(b_attn, dtype=np.float32)
    W_proj_f = np.asarray(W_proj, dtype=np.float32)
    b_proj_f = np.asarray(b_proj, dtype=np.float32)

    W_attn_b = np.ascontiguousarray(W_attn_f.astype(bf))
    W_proj_b = np.ascontiguousarray(W_proj_f.astype(bf))
    # q-bias as per-partition columns: col hp = [bq[2hp] | bq[2hp+1]]
    bq = b_attn_f[:H].reshape(NH, HS)
    bqc = np.empty((128, NP), dtype=np.float32)
    for hp in range(NP):
        bqc[:64, hp] = bq[2 * hp]
        bqc[64:, hp] = bq[2 * hp + 1]
    bqc = np.ascontiguousarray(bqc)
    # v-bias folded into projection bias (exact: sum_k P = 1 per row)
    bp_eff = np.ascontiguousarray(
        (b_attn_f[2 * H:] @ W_proj_f + b_proj_f).astype(bf))
    return [
        {
            "x": np.ascontiguousarray(x[b]),
            "W_attn": W_attn_b,
            "bq": bqc,
            "W_proj": W_proj_b,
            "bp": bp_eff,
        }
        for b in range(N_CORES)
    ]


def kernel(x, W_attn, b_attn, W_proj, b_proj):
    in_maps = make_in_maps(x, W_attn, b_attn, W_proj, b_proj)
    res = _run(in_maps, trace=False)
    return np.stack([res.results[b]["y"] for b in range(N_CORES)]).astype(np.float32)



# revision 51
# speedup vs baseline: 1.0978x; 1.0978x over previous
"""Multi-head causal self-attention (GPT-style block) on 8 Trainium2 NeuronCores.

Data-parallel over batch (B=8 -> 1 element/core), weights replicated.

v3 design (v2 measured 181.8us):
- Bias algebra: k-bias dropped (softmax row-shift invariance), v-bias folded
  into b_proj on host, q-bias added per-partition in the q evacuation.
- Scores are K=128 with the inactive half of qTp zeroed (true K=64 PE row
  tiling via tile_position miscompiles on HW: CoreSim-correct, garbage on
  device -- do not retry).
- Fast normalization: av psum evacuated to SBUF in one fp32 copy (frees the
  bank that gates the next head pair's AV), reciprocal of just the ones-row,
  bf16 cast, rank-1 matmul broadcast of 1/d, one DVE mul av_sb*bp -> oT
  (single psum operand; two-psum-operand TensorTensor is rejected by the
  BIR verifier, as are partition-broadcast DMAs and partition-strided ops).
- ScalarE: exp + a few early evacs; DVE: psum evacs/normalize; Pool: masks.
- qg0+qg1 attention interleaved per head pair with qkv/proj matmuls as PE
  filler; PE warm-up junk matmuls + multi-queue DMA issue shrink the head.
"""

import numpy as np

import concourse.bass as bass
import concourse.mybir as mybir
import concourse.tile as tile
from concourse import bacc, bass_utils
from concourse.masks import make_identity, make_upper_triangular

F32 = mybir.dt.float32
F32R = mybir.dt.float32r
BF16 = mybir.dt.bfloat16
EXP = mybir.ActivationFunctionType.Exp

T = 1024
H = 768
NH = 12
HS = 64
TT = T // 128   # 8 token tiles
FT = H // 128   # 6 feature tiles
NP = NH // 2    # 6 head pairs
N_CORES = 8

PAIRS = {0: [(0, 1), (2, 3)], 1: [(0, 1), (2, 3), (4, 5), (6, 7)]}


def _pair_geom(qg, kt0, kt1):
    off0 = max(128 * kt0, 512 * qg)
    off1 = max(128 * kt1, 512 * qg)
    return off0, 512 * (qg + 1) - off0, off1, 512 * (qg + 1) - off1


def build():
    nc = bacc.Bacc(None, target_bir_lowering=False)

    x_d = nc.dram_tensor("x", [T, H], BF16, kind="ExternalInput")
    wa_d = nc.dram_tensor("W_attn", [H, 3 * H], BF16, kind="ExternalInput")
    bq_d = nc.dram_tensor("bq", [128, NP], F32, kind="ExternalInput")
    wp_d = nc.dram_tensor("W_proj", [H, H], BF16, kind="ExternalInput")
    bp_d = nc.dram_tensor("bp", [H], BF16, kind="ExternalInput")
    y_d = nc.dram_tensor("y", [T, H], BF16, kind="ExternalOutput")

    with tile.TileContext(nc) as tc:
        with (
            tc.tile_pool(name="sb", bufs=1) as sb,
            tc.tile_pool(name="ps", bufs=1, space="PSUM") as ps,
        ):
            # ---------------- persistent SBUF ----------------
            wat = sb.tile([128, FT, 3 * H], BF16, tag="wat")
            wpr = sb.tile([128, FT, H], BF16, tag="wpr")
            x_bf = sb.tile([128, TT, H], BF16, tag="x_bf")
            xT = sb.tile([128, FT, T], BF16, tag="xT")
            qTp = sb.tile([128, NH, T], BF16, tag="qTp")
            kT = sb.tile([128, NP, T], BF16, tag="kT")
            v_bf = sb.tile([128, TT, NH * (HS + 1) + 64], BF16, tag="v_bf")
            oT = sb.tile([128, FT, T], BF16, tag="oT")
            bqc = sb.tile([128, NP], F32, tag="bqc")
            bp_rowb = sb.tile([1, H], BF16, tag="bp_rowb")
            ones0 = sb.tile([1, 128], BF16, tag="ones0")
            ones_all = sb.tile([65, 64], BF16, tag="ones_all")  # row 64 = 1.0
            tri = sb.tile([128, 128], BF16, tag="tri")
            ident = sb.tile([128, 128], BF16, tag="ident")
            warm = sb.tile([128, 512], BF16, tag="warm")
            junks = sb.tile([1, 16], F32, tag="junks")

            # ---------------- consts (pool engine, front of its queue) ----
            nc.vector.memset(warm[:], 0.125)
            make_identity(nc, ident[:])
            make_upper_triangular(nc, tri[:], val=1.0, diag=True)
            nc.gpsimd.memset(ones0[:], 1.0)
            nc.gpsimd.memset(ones_all[:], 1.0)
            nc.gpsimd.memset(v_bf[:, :, NH * (HS + 1):], 0.0)
            nc.gpsimd.memset(v_bf[:, :, HS:NH * (HS + 1):HS + 1], 1.0)
            nc.gpsimd.memset(qTp[64:128, 0:NH:2, :], 0.0)
            nc.gpsimd.memset(qTp[0:64, 1:NH:2, :], 0.0)

            # ---------------- PE warm-up (HAM) while DMAs land ------------
            with nc.named_scope("head"):
                pw = ps.tile([128, 512], F32, tag="av", bufs=2, name="pw")
                for _ in range(24):
                    nc.tensor.matmul(pw[:], warm[:, :128], warm[:], start=True, stop=True)
                # ACT exp table preload
                nc.scalar.activation(junks[:], warm[:1, :16], EXP, scale=0.125)

            # ---------------- DMA issue, spread across queues -------------
            # priority: x tiles 0-3 and the qk weights gate the first scores;
            # x tiles 4-7 are only needed for the second token group.
            for tt in range(4):
                eng = nc.sync if tt % 2 == 0 else nc.scalar
                eng.dma_start(x_bf[:, tt, :], x_d[tt * 128:(tt + 1) * 128, :])
            for ft in range(FT):
                eng = nc.scalar if ft % 2 == 0 else nc.sync
                eng.dma_start(wat[:, ft, :2 * H], wa_d[ft * 128:(ft + 1) * 128, :2 * H])
            for tt in range(4, TT):
                eng = nc.sync if tt % 2 == 0 else nc.scalar
                eng.dma_start(x_bf[:, tt, :], x_d[tt * 128:(tt + 1) * 128, :])
            nc.gpsimd.dma_start(bqc[:], bq_d[:, :])
            for ft in range(FT):
                nc.gpsimd.dma_start(wat[:, ft, 2 * H:], wa_d[ft * 128:(ft + 1) * 128, 2 * H:])
            for ft in range(FT):
                nc.gpsimd.dma_start(wpr[:, ft, :], wp_d[ft * 128:(ft + 1) * 128, :])
            nc.gpsimd.dma_start(bp_rowb[:], bp_d[None, :])

            # ---------------- emission helpers ----------------------------
            def emit_qk(hp, which, tgs=(0, 1)):
                """q (nt=hp) or k (nt=6+hp) projection -> qTp/kT, DVE evac.
                q-bias is fused into the DVE evac as a per-partition add."""
                nt = hp if which == "q" else NP + hp
                for tg in tgs:
                    pq = ps.tile([128, 512], F32, tag="op", bufs=2, name="pq")
                    for ft in range(FT):
                        nc.tensor.matmul(
                            pq[:],
                            wat[:, ft, nt * 128:(nt + 1) * 128],
                            xT[:, ft, tg * 512:(tg + 1) * 512],
                            start=(ft == 0),
                            stop=(ft == FT - 1),
                        )
                    sl = slice(tg * 512, (tg + 1) * 512)
                    if which == "q":
                        nc.vector.tensor_scalar_add(
                            qTp[0:64, 2 * hp, sl], pq[0:64, :], bqc[0:64, hp:hp + 1])
                        nc.vector.tensor_scalar_add(
                            qTp[64:128, 2 * hp + 1, sl], pq[64:128, :],
                            bqc[64:128, hp:hp + 1])
                    else:
                        nc.vector.tensor_copy(kT[:, hp, sl], pq[:])

            def emit_vmm(tt, early):
                """v projection for one token tile -> v_pl."""
                for ng in range(2):
                    pv = ps.tile([128, 512], F32, tag="op", bufs=2, name="pv")
                    for ft in range(FT):
                        nc.tensor.matmul(
                            pv[:, :384],
                            xT[:, ft, tt * 128:(tt + 1) * 128],
                            wat[:, ft, 2 * H + 384 * ng: 2 * H + 384 * (ng + 1)],
                            start=(ft == 0),
                            stop=(ft == FT - 1),
                        )
                    dst = v_bf[:, tt, 390 * ng:390 * (ng + 1)].rearrange(
                        "p (h d) -> p h d", d=65)[:, :, :64]
                    srcv = pv[:, :384].rearrange("p (h d) -> p h d", d=64)
                    # early tiles: ACT is idle pre-attn; later ones: DVE
                    if early:
                        nc.scalar.copy(dst, srcv)
                    else:
                        nc.vector.tensor_copy(dst, srcv)

            def emit_scores_pair(hp, qg, pi, pts):
                """scores + exp + mask for one kt pair. Both heads' scores
                land in ONE psum tile so exp is a single wide ACT instruction
                and each mask mul covers both heads."""
                kt0, kt1 = PAIRS[qg][pi]
                off0, w0, off1, w1 = _pair_geom(qg, kt0, kt1)
                sps = [
                    ps.tile([128, 1024], F32, tag=f"s{hi}", bufs=1, name=f"sp{hi}")
                    for hi in range(2)
                ]
                for kt, off, w, so in ((kt0, off0, w0, 0), (kt1, off1, w1, w0)):
                    for hi in range(2):
                        nc.tensor.matmul(
                            sps[hi][:, so:so + w],
                            kT[:, hp, kt * 128:(kt + 1) * 128],
                            qTp[:, 2 * hp + hi, off:off + w],
                            start=True,
                            stop=True,
                        )
                vw = w0 + w1
                diag = 128 * kt0 >= 512 * qg
                for hi in range(2):
                    nc.scalar.activation(
                        pts[:, pi, hi, :vw], sps[hi][:, :vw], EXP, scale=0.125)
                    if diag:
                        nc.gpsimd.tensor_mul(
                            pts[:, pi, hi, :128], pts[:, pi, hi, :128],
                            tri[:])
                        nc.gpsimd.tensor_mul(
                            pts[:, pi, hi, w0:w0 + 128],
                            pts[:, pi, hi, w0:w0 + 128], tri[:])

            def emit_av(hp, qg, pts, avs, pis, first, last):
                """AV accumulation for pair indices pis of (hp, qg)."""
                for hi in range(2):
                    h = 2 * hp + hi
                    for pi in pis:
                        kt0, kt1 = PAIRS[qg][pi]
                        off0, w0, off1, w1 = _pair_geom(qg, kt0, kt1)
                        for kt, off, w, so in ((kt0, off0, w0, 0), (kt1, off1, w1, w0)):
                            nc.tensor.matmul(
                                avs[hi][:65, off - 512 * qg: off - 512 * qg + w],
                                v_bf[:, kt, 65 * h:65 * h + 65],
                                pts[:, pi, hi, so:so + w],
                                start=(first and pi == pis[0] and so == 0),
                                stop=(last and pi == pis[-1] and so == w0),
                            )

            def emit_recip(avs, on_act=False):
                """Evacuate av psum -> fp32 SBUF (frees the bank fast, it
                gates the next head pair's AV), reciprocal of the whole tile
                (a base-64 single-partition custom DVE op NaNs on HW; rows
                0-63 are unused junk), bf16 cast of the denominator row.
                on_act moves the copies to ACT (used in the flush, where ACT
                has no exp work and the DVE is the serializer)."""
                avsbs = []
                recbs = []
                for hi in range(2):
                    avsb = sb.tile([65, 512], F32, tag="avsb", bufs=4, name="avsb")
                    rec = sb.tile([65, 512], F32, tag="rec", bufs=3, name="rec")
                    recb = sb.tile([65, 512], BF16, tag="recb", bufs=3, name="recb")
                    if on_act:
                        nc.scalar.copy(avsb[:, :], avs[hi][:65, :])
                    else:
                        nc.vector.tensor_copy(avsb[:, :], avs[hi][:65, :])
                    nc.vector.reciprocal_approx_fast(rec[:, :], avsb[:, :])
                    if on_act:
                        nc.scalar.copy(recb[64:65, :], rec[64:65, :])
                    else:
                        nc.vector.tensor_copy(recb[64:65, :], rec[64:65, :])
                    avsbs.append(avsb)
                    recbs.append(recb)
                return (avsbs, recbs)

            def emit_norm(hp, qg, avs, r, hi):
                """broadcast 1/d to 64 partitions with a bf16 rank-1 matmul,
                then one DVE mul av_sb * bp -> oT (single psum operand)."""
                avsbs, recbs = r
                bp = ps.tile([128, 512], F32, tag="op", bufs=2, name="bp")
                nc.tensor.matmul(
                    bp[:64, :],
                    ones_all[64:65, :64],
                    recbs[hi][64:65, :],
                    start=True, stop=True,
                )
                dst = slice(512 * qg, 512 * (qg + 1))
                if hi == 0:
                    nc.vector.tensor_mul(oT[:64, hp, dst], avsbs[0][:64, :], bp[:64, :])
                else:
                    sc = sb.tile([64, 512], BF16, tag="sc", bufs=4, name="sc")
                    nc.vector.tensor_mul(sc[:], avsbs[1][:64, :], bp[:64, :])
                    nc.sync.dma_start(oT[64:128, hp, dst], sc[:])

            def emit_proj(tt):
                ysb = sb.tile([128, H], BF16, tag="ysb", bufs=4, name="ysb")
                for ng in range(2):
                    py = ps.tile([128, 512], F32, tag="op", bufs=2, name="py")
                    for ft in range(FT):
                        nc.tensor.matmul(
                            py[:, :384],
                            oT[:, ft, tt * 128:(tt + 1) * 128],
                            wpr[:, ft, 384 * ng:384 * (ng + 1)],
                            start=(ft == 0),
                            stop=False,
                        )
                    nc.tensor.matmul(
                        py[:, :384],
                        ones0[:1, :],
                        bp_rowb[:1, 384 * ng:384 * (ng + 1)],
                        start=False,
                        stop=True,
                    )
                    nc.scalar.copy(ysb[:, 384 * ng:384 * (ng + 1)], py[:, :384])
                    nc.sync.dma_start(
                        y_d[tt * 128:(tt + 1) * 128, 384 * ng:384 * (ng + 1)],
                        ysb[:, 384 * ng:384 * (ng + 1)])

            # ---------------- x transpose (evac on DVE) -------------------
            # qk for token group 0 only needs x tiles 0-3: emit it mid-loop
            # so the first scores can start while x tiles 4-7 transpose.
            with nc.named_scope("xT"):
                for tt in range(TT):
                    pt = ps.tile([128, FT * 128], BF16, tag="op", bufs=2, name="pt")
                    for ft in range(FT):
                        nc.tensor.transpose(
                            pt[:, ft * 128:(ft + 1) * 128],
                            x_bf[:, tt, ft * 128:(ft + 1) * 128],
                            ident[:],
                        )
                    if tt % 2 == 0:
                        nc.vector.tensor_copy(
                            xT[:, :, tt * 128:(tt + 1) * 128],
                            pt[:].rearrange("p (f t) -> p f t", t=128),
                        )
                    else:
                        nc.scalar.copy(
                            xT[:, :, tt * 128:(tt + 1) * 128],
                            pt[:].rearrange("p (f t) -> p f t", t=128),
                        )
                    if tt == 3:
                        emit_qk(0, "q", tgs=(0,))
                        emit_qk(0, "k", tgs=(0,))

            # ---------------- main pipeline -------------------------------
            # step hp: scores(hp) both qg, AV/recip/norm(hp-1), interleaved
            # with qk/t/v/etapply fillers between dependent chunks.
            with nc.named_scope("attn"):
                state = {}
                for hp in range(NP + 1):
                    prev = state.get(hp - 1)
                    if prev is not None:
                        prev["av0"] = [
                            ps.tile([128, 512], F32, tag="av", bufs=2, name="av0")
                            for _ in range(2)
                        ]
                    if hp < NP:
                        pts0 = sb.tile([128, 2, 2, 1024], BF16, tag="pA", bufs=2,
                                       name="ptsA")
                        pts = sb.tile([128, 4, 2, 1024], BF16, tag="p", bufs=2,
                                      name="pts")
                        state[hp] = {"pts": pts, "pts0": pts0}
                        if prev is not None:
                            emit_av(hp - 1, 0, prev["pts0"], prev["av0"], [0],
                                    first=True, last=False)
                        prev2 = state.get(hp - 2)
                        if prev2 is not None:
                            emit_norm(hp - 2, 1, prev2["av1"], prev2["r1"], 0)
                            emit_norm(hp - 2, 1, prev2["av1"], prev2["r1"], 1)
                        emit_scores_pair(hp, 0, 0, pts0)
                        if hp == 0:
                            emit_qk(0, "q", tgs=(1,))
                            emit_vmm(0, early=False)
                            emit_vmm(1, early=False)
                        if prev is not None:
                            emit_av(hp - 1, 0, prev["pts0"], prev["av0"], [1],
                                    first=False, last=True)
                        emit_scores_pair(hp, 0, 1, pts0)
                        if hp == 0:
                            emit_qk(0, "k", tgs=(1,))
                            emit_vmm(2, early=False)
                            emit_vmm(3, early=False)
                        if prev is not None:
                            prev["r0"] = emit_recip(prev["av0"])
                            prev["av1"] = [
                                ps.tile([128, 512], F32, tag="av", bufs=2, name="av1")
                                for _ in range(2)
                            ]
                        emit_scores_pair(hp, 1, 0, pts)
                        if hp == 0:
                            for tt in range(4, TT):
                                emit_vmm(tt, early=False)
                        if prev is not None:
                            emit_av(hp - 1, 1, prev["pts"], prev["av1"], [0],
                                    first=True, last=False)
                            emit_norm(hp - 1, 0, prev["av0"], prev["r0"], 0)
                        emit_scores_pair(hp, 1, 1, pts)
                        if prev is not None:
                            emit_av(hp - 1, 1, prev["pts"], prev["av1"], [1],
                                    first=False, last=False)
                            emit_norm(hp - 1, 0, prev["av0"], prev["r0"], 1)
                        emit_scores_pair(hp, 1, 2, pts)
                        if hp + 1 < NP:
                            emit_qk(hp + 1, "q")
                        if prev is not None:
                            emit_av(hp - 1, 1, prev["pts"], prev["av1"], [2],
                                    first=False, last=False)
                        emit_scores_pair(hp, 1, 3, pts)
                        if hp + 1 < NP:
                            emit_qk(hp + 1, "k")
                        if prev is not None:
                            emit_av(hp - 1, 1, prev["pts"], prev["av1"], [3],
                                    first=False, last=True)
                            prev["r1"] = emit_recip(prev["av1"])
                    else:
                        # flush last head pair: keep the PE stream dense
                        # (av0, av1, proj0..3) while the cheap norm chains
                        # drain on DVE in parallel.
                        prev2 = state.get(hp - 2)
                        emit_av(hp - 1, 0, prev["pts0"], prev["av0"], [0, 1],
                                first=True, last=True)
                        prev["r0"] = emit_recip(prev["av0"], on_act=True)
                        if prev2 is not None:
                            emit_norm(hp - 2, 1, prev2["av1"], prev2["r1"], 0)
                            emit_norm(hp - 2, 1, prev2["av1"], prev2["r1"], 1)
                        prev["av1"] = [
                            ps.tile([128, 512], F32, tag="av", bufs=2, name="av1")
                            for _ in range(2)
                        ]
                        emit_av(hp - 1, 1, prev["pts"], prev["av1"], [0, 1],
                                first=True, last=False)
                        emit_norm(hp - 1, 0, prev["av0"], prev["r0"], 0)
                        emit_norm(hp - 1, 0, prev["av0"], prev["r0"], 1)
                        emit_av(hp - 1, 1, prev["pts"], prev["av1"], [2, 3],
                                first=False, last=True)
                        prev["r1"] = emit_recip(prev["av1"], on_act=True)
                        emit_proj(0)
                        emit_proj(1)
                        emit_norm(hp - 1, 1, prev["av1"], prev["r1"], 0)
                        emit_norm(hp - 1, 1, prev["av1"], prev["r1"], 1)
                        emit_proj(2)
                        emit_proj(3)

            # ---------------- output projection ---------------------------
            with nc.named_scope("proj"):
                for tt in range(4, TT):
                    emit_proj(tt)

    nc.compile()
    return nc


_NC = None


def _run(in_maps, trace=False, **kwargs):
    global _NC
    if _NC is None:
        _NC = build()
    return bass_utils.run_bass_kernel_spmd(
        _NC, in_maps, core_ids=list(range(N_CORES)), trace=trace, **kwargs
    )


def make_in_maps(x, W_attn, b_attn, W_proj, b_proj):
    import ml_dtypes
    bf = ml_dtypes.bfloat16
    x = np.asarray(x, dtype=np.float32).astype(bf)
    W_attn_f = np.asarray(W_attn, dtype=np.float32)
    b_attn_f = np.asarray(b_attn, dtype=np.float32)
    W_proj_f = np.asarray(W_proj, dtype=np.float32)
    b_proj_f = np.asarray(b_proj, dtype=np.float32)

    W_attn_b = np.ascontiguousarray(W_attn_f.astype(bf))
    W_proj_b = np.ascontiguousarray(W_proj_f.astype(bf))
    # q-bias as per-partition columns: col hp = [bq[2hp] | bq[2hp+1]]
    bq = b_attn_f[:H].reshape(NH, HS)
    bqc = np.empty((128, NP), dtype=np.float32)
    for hp in range(NP):
        bqc[:64, hp] = bq[2 * hp]
        bqc[64:, hp] = bq[2 * hp + 1]
    bqc = np.ascontiguousarray(bqc)
    # v-bias folded into projection bias (exact: sum_k P = 1 per row)
    bp_eff = np.ascontiguousarray(
        (b_attn_f[2 * H:] @ W_proj_f + b_proj_f).astype(bf))
    return [
        {
            "x": np.ascontiguousarray(x[b]),
            "W_attn": W_attn_b,
            "bq": bqc,
            "W_proj": W_proj_b,
            "bp": bp_eff,
        }
        for b in range(N_CORES)
    ]


def kernel(x, W_attn, b_attn, W_proj, b_proj):
    in_maps = make_in_maps(x, W_attn, b_attn, W_proj, b_proj)
    res = _run(in_maps, trace=False)
    return np.stack([res.results[b]["y"] for b in range(N_CORES)]).astype(np.float32)

